# revision 13
# baseline (speedup 1.0000x reference)
"""Trainium2 Bass kernel for the 4-layer quantized MLP (dense_mlp).

Strategy
--------
Data-parallel over the batch dim: each of the 8 cores processes 1024 of the
8192 rows with the full set of weights (no collectives; host gathers).

Numerics: quant_weight() yields integer codes q in {-3..3} times one fp32
scale, and quant_relu() (with the given act scales) yields integer
activations in {0..15}.  Both are exactly representable in fp8e4m3, so
layers 2-4 run as exact integer arithmetic in fp8 with DoubleRow perf mode
(fp32 PSUM accumulation is exact: |partial sums| < 2^19).  Layer 1 streams
the continuous fp32 input as an fp16 hi+lo split (two fp16 matmuls
accumulating into the same PSUM bank), giving fp32-level precision at the
fp16 matmul rate.  Per-layer affine (weight scale x BN fold / act scale) is
applied on eviction: ACT does relu(z*alpha+beta), DVE does min(.,15) and
round-to-nearest-even via the +-2^23 trick, casting to fp8 for the next
layer.  All activations stay resident in SBUF between layers.

Layout: activations are kept feature-major [128, K/128, M] so each layer's
PSUM output tile ([h_tile partitions] x [batch free dim]) is directly the
next layer's contraction operand - no transposes anywhere on device.

Startup: a short PE warmup un-throttles the HAM clock gate, then the first
PRO h-tiles of each M-half run hi(xh)-only sweeps chunk-outer across PRO
open PSUM banks, with their weight DMAs interleaved into the x chunk
stream.  This keeps the PE dense while x streams from HBM, both at kernel
start and at the mh=0 -> mh=1 transition.

Perf notes (measured): every 512-FD matmul costs ~518 PE cycles regardless
of dtype/perf-mode (fp8 DoubleRow's win is 2x K per instruction, not
faster rows), LDWEIGHTS is hidden by the background weight buffer, and the
kernel is ~99.5% PE-bound at the 16896-slot floor.  Layer 1 cannot drop
below 2 passes: the quantization boundaries amplify input error ~sqrt per
stage (one flipped h1 element corrupts its whole output row), so x needs
>=20 bits, and no operand encoding beats fp16's 11 bits/slice/slot.
"""

import os
import sys

import numpy as np

# The Bass kernel runs through jax/PJRT on the neuron (axon) backend. If the
# caller pinned JAX_PLATFORMS=cpu (common for running the pure-jax reference)
# and jax has not been imported yet, lift the pin so the devices are visible.
if os.environ.get("JAX_PLATFORMS") == "cpu" and "jax" not in sys.modules:
    os.environ["JAX_PLATFORMS"] = ""

B, D, H, C = 8192, 4096, 8192, 1000
NCORES = 8
M = B // NCORES          # 1024 batch rows per core
P = 128
CPAD = 1024              # padded output features (w4 zero-padded 1000->1024)
C23 = float(2.0 ** 23)   # RNE integer-rounding constant for fp32

f32 = np.float32


def _quant_int(w):
    """Integer weight codes + scale, replicating quant_weight() in fp32."""
    w = np.ascontiguousarray(w, dtype=f32)
    scale = (np.max(np.abs(w)) / f32(3.0)).astype(f32)
    q = np.round(np.clip((w / scale).astype(f32), f32(-3.0), f32(3.0))).astype(f32)
    return q, scale


def _feat_major(a, ksub):
    """[K, N] -> [128, ksub, N] with k = ks*128 + p."""
    K, N = a.shape
    assert K == ksub * P
    return np.ascontiguousarray(a.reshape(ksub, P, N).transpose(1, 0, 2))


def _w_prep(q, ksub, htiles, wdt):
    """q [Hout, K] -> [128, htiles, ksub, 128]: w[p, ht, ks, hh] = q[ht*128+hh, ks*128+p]."""
    Hout, K = q.shape
    assert Hout == htiles * P and K == ksub * P
    t = q.T.reshape(ksub, P, htiles, P).transpose(1, 2, 0, 3)
    return np.ascontiguousarray(t).astype(wdt)


def _per_part(v, ntiles):
    """[ntiles*128] -> [128, ntiles] with v[p, t] = v[t*128+p]."""
    return np.ascontiguousarray(v.reshape(ntiles, P).T, dtype=f32)


def _build_bass():
    import concourse.bacc as bacc
    import concourse.tile as tile
    from concourse import mybir
    from contextlib import ExitStack

    fp16 = mybir.dt.float16
    fp8 = mybir.dt.float8e4
    fp32 = mybir.dt.float32
    DR = mybir.MatmulPerfMode.DoubleRow
    Relu = mybir.ActivationFunctionType.Relu
    op = mybir.AluOpType

    KS1 = D // P          # 32  k-subtiles for layer 1
    KS = H // P           # 64  k-subtiles for layers 2-4
    HT = H // P           # 64  h-tiles for layers 1-3
    HT4 = CPAD // P       # 8   h-tiles for layer 4

    nc = bacc.Bacc(None, target_bir_lowering=False)

    xh_d = nc.dram_tensor("xh", [P, KS1, M], fp16, kind="ExternalInput")
    xl_d = nc.dram_tensor("xl", [P, KS1, M], fp16, kind="ExternalInput")
    w1_d = nc.dram_tensor("w1q", [P, HT, KS1, P], fp16, kind="ExternalInput")
    w2_d = nc.dram_tensor("w2q", [P, HT, KS, P], fp8, kind="ExternalInput")
    w3_d = nc.dram_tensor("w3q", [P, HT, KS, P], fp8, kind="ExternalInput")
    w4_d = nc.dram_tensor("w4q", [P, HT4, KS, P], fp8, kind="ExternalInput")
    ab_d = {}
    for i in (1, 2, 3):
        ab_d[f"al{i}"] = nc.dram_tensor(f"al{i}", [P, HT], fp32, kind="ExternalInput")
        ab_d[f"be{i}"] = nc.dram_tensor(f"be{i}", [P, HT], fp32, kind="ExternalInput")
    ab_d["al4"] = nc.dram_tensor("al4", [P, HT4], fp32, kind="ExternalInput")
    out_d = nc.dram_tensor("out", [P, HT4, M], fp32, kind="ExternalOutput")

    with tile.TileContext(nc) as tc, ExitStack() as ctx:
        const = ctx.enter_context(tc.tile_pool(name="const", bufs=1))
        acts = ctx.enter_context(tc.tile_pool(name="acts", bufs=1))
        wp = ctx.enter_context(tc.tile_pool(name="wp", bufs=6))
        pp = ctx.enter_context(tc.tile_pool(name="pp", bufs=7, space="PSUM"))
        tp = ctx.enter_context(tc.tile_pool(name="tp", bufs=4))
        ost = ctx.enter_context(tc.tile_pool(name="ost", bufs=2))

        # PE warmup: tiny matmuls on zeroed SBUF un-throttle the HAM clock
        # gate (~3.4us of sustained activity needed) while the first input
        # DMAs are in flight, so the real matmuls start warm at 2.4 GHz.
        wdum = const.tile([P, P], fp16, name="wdum")
        rdum = const.tile([P, P], fp16, name="rdum")
        nc.vector.memset(wdum, 0.0)
        nc.vector.memset(rdum, 0.0)
        warm_ps = pp.tile([P, P], fp32, tag="warm", name="warm_ps", bufs=1)
        for wi in range(64):
            nc.tensor.matmul(warm_ps[:, :], wdum[:, :], rdum[:, :],
                             start=True, stop=True)

        # chunked x loads: range-based overlap tracking lets matmuls start as
        # soon as the chunk they read has landed.  The first PRO h-tiles'
        # weight DMAs are interleaved into the chunk stream just ahead of
        # when the PE needs them.
        PRO = 6
        xchunks = [(0, 2), (2, 2), (4, 4), (8, 8), (16, 8), (24, 8)]

        def issue_x_and_w(mh):
            ms = slice(mh * 512, (mh + 1) * 512)
            xh_t = acts.tile([P, KS1, 512], fp16, tag="bufA", name=f"xh_{mh}")
            xl_t = acts.tile([P, KS1, 512], fp16, tag="bufB", name=f"xl_{mh}")
            wts = []
            for i, (kc, kn) in enumerate(xchunks):
                if i < PRO:
                    wt = wp.tile([P, KS1, P], fp16, tag="wt", name=f"w1_{mh}_{i}")
                    nc.sync.dma_start(out=wt, in_=w1_d[:, i, :, :])
                    wts.append(wt)
                nc.sync.dma_start(out=xh_t[:, kc:kc + kn, :],
                                  in_=xh_d[:, kc:kc + kn, ms])
            for kc, kn in xchunks:
                nc.sync.dma_start(out=xl_t[:, kc:kc + kn, :],
                                  in_=xl_d[:, kc:kc + kn, ms])
            return xh_t, xl_t, wts

        # mh=0 x data goes on the queue ahead of the (small, not yet needed)
        # affine constants so the first matmuls aren't DMA-gated
        x_pre = issue_x_and_w(0)

        ab = {}
        for name, d in ab_d.items():
            t = const.tile(list(d.shape), fp32, name=f"c_{name}")
            nc.sync.dma_start(out=t, in_=d[:])
            ab[name] = t

        def evict_quant(psum, al, be, ht, dst):
            """dst[:, :] = fp8(round(clip(relu(psum*al+be), 0, 15)))"""
            u = tp.tile([P, 512], fp32, tag="u", name="u")
            nc.scalar.activation(u, psum[:, :], Relu, bias=be, scale=al)
            v = tp.tile([P, 512], fp32, tag="v", name="v")
            nc.vector.tensor_scalar(v, u, 15.0, C23, op0=op.min, op1=op.add)
            nc.vector.tensor_scalar(dst, v, C23, None, op0=op.subtract)

        # ---- Layer 1: z1 = x @ q1.T via fp16 hi+lo, M in two halves ----
        # Layer-1 loop: the first PRO h-tiles of each M-half run their hi
        # (xh-only) sweeps back-to-back before any lo sweep, into PRO
        # concurrently-open PSUM banks.  The hi sweeps only need xh, which is
        # first in the DMA queue, so the PE stays busy while xl (and at mh=1,
        # the whole x half) is still streaming in.
        a1 = acts.tile([P, HT, M], fp8, tag="bufC", name="a1")
        for mh in range(2):
            ms = slice(mh * 512, (mh + 1) * 512)
            xh_t, xl_t, pro_wt = x_pre if mh == 0 else issue_x_and_w(1)
            pro_ps = [pp.tile([P, 512], fp32, tag="ps", name=f"ps1_{mh}_{ht}")
                      for ht in range(PRO)]
            # chunk-outer consumption: each landed x chunk feeds all PRO
            # h-tiles, so the PE eats chunks faster than HBM delivers them
            # and never bubbles waiting for the tail of the x stream.
            for xt, is_lo in ((xh_t, False), (xl_t, True)):
                for kc, kn in xchunks:
                    for ht in range(PRO):
                        for ks in range(kc, kc + kn):
                            nc.tensor.matmul(
                                pro_ps[ht][:, :], pro_wt[ht][:, ks, :],
                                xt[:, ks, :],
                                start=(not is_lo and ks == 0),
                                stop=(is_lo and ks == KS1 - 1))
            for ht in range(PRO):
                evict_quant(pro_ps[ht], ab["al1"][:, ht:ht + 1],
                            ab["be1"][:, ht:ht + 1], ht, a1[:, ht, ms])
            for ht in range(PRO, HT):
                wt = wp.tile([P, KS1, P], fp16, tag="wt", name=f"w1_{mh}_{ht}")
                nc.sync.dma_start(out=wt, in_=w1_d[:, ht, :, :])
                ps = pp.tile([P, 512], fp32, tag="ps", name=f"ps1_{mh}_{ht}")
                for ks in range(KS1):
                    nc.tensor.matmul(ps[:, :], wt[:, ks, :], xh_t[:, ks, :],
                                     start=(ks == 0), stop=False)
                    nc.tensor.matmul(ps[:, :], wt[:, ks, :], xl_t[:, ks, :],
                                     start=False, stop=(ks == KS1 - 1))
                evict_quant(ps, ab["al1"][:, ht:ht + 1], ab["be1"][:, ht:ht + 1],
                            ht, a1[:, ht, ms])

        # ---- Layers 2-3: fp8 DoubleRow, output split across two 32KB tiles ----
        def mid_layer(idx, rhs_parts, w_d, al, be):
            # rhs_parts: list of SBUF tiles [P, 32, M] covering ks 0..63
            outs = [acts.tile([P, KS // 2, M], fp8, tag=t, name=f"a{idx}{t}")
                    for t in ("bufA", "bufB")]
            for ht in range(HT):
                wt = wp.tile([P, KS, P], fp8, tag="wt", name=f"w{idx}_{ht}")
                nc.sync.dma_start(out=wt, in_=w_d[:, ht, :, :])
                pss = [pp.tile([P, 512], fp32, tag="ps", name=f"ps{idx}_{ht}_{i}")
                       for i in range(2)]
                for ki, ks in enumerate(range(0, KS, 2)):
                    part, kk = rhs_parts[ks // 32], ks % 32
                    for mb in range(2):
                        nc.tensor.matmul(
                            pss[mb][:, :], wt[:, ks:ks + 2, :],
                            part[:, kk:kk + 2, mb * 512:(mb + 1) * 512],
                            start=(ki == 0), stop=(ki == KS // 2 - 1),
                            perf_mode=DR)
                dst = outs[ht // 32]
                for mb in range(2):
                    evict_quant(pss[mb], al[:, ht:ht + 1], be[:, ht:ht + 1],
                                ht, dst[:, ht % 32, mb * 512:(mb + 1) * 512])
            return outs

        a2 = mid_layer(2, [a1[:, :KS // 2, :], a1[:, KS // 2:, :]],
                       w2_d, ab["al2"], ab["be2"])
        a3 = acts.tile([P, HT, M], fp8, tag="bufC", name="a3")
        for ht in range(HT):
            wt = wp.tile([P, KS, P], fp8, tag="wt", name=f"w3_{ht}")
            nc.sync.dma_start(out=wt, in_=w3_d[:, ht, :, :])
            pss = [pp.tile([P, 512], fp32, tag="ps", name=f"ps3_{ht}_{i}")
                   for i in range(2)]
            for ki, ks in enumerate(range(0, KS, 2)):
                part, kk = a2[ks // 32], ks % 32
                for mb in range(2):
                    nc.tensor.matmul(
                        pss[mb][:, :], wt[:, ks:ks + 2, :],
                        part[:, kk:kk + 2, mb * 512:(mb + 1) * 512],
                        start=(ki == 0), stop=(ki == KS // 2 - 1),
                        perf_mode=DR)
            for mb in range(2):
                evict_quant(pss[mb], ab["al3"][:, ht:ht + 1], ab["be3"][:, ht:ht + 1],
                            ht, a3[:, ht, mb * 512:(mb + 1) * 512])

        # ---- Layer 4: out = (a3 @ q4.T) * (s4 * act_scale3) ----
        for ht in range(HT4):
            wt = wp.tile([P, KS, P], fp8, tag="wt", name=f"w4_{ht}")
            nc.sync.dma_start(out=wt, in_=w4_d[:, ht, :, :])
            pss = [pp.tile([P, 512], fp32, tag="ps", name=f"ps4_{ht}_{i}")
                   for i in range(2)]
            for ki, ks in enumerate(range(0, KS, 2)):
                for mb in range(2):
                    nc.tensor.matmul(
                        pss[mb][:, :], wt[:, ks:ks + 2, :],
                        a3[:, ks:ks + 2, mb * 512:(mb + 1) * 512],
                        start=(ki == 0), stop=(ki == KS // 2 - 1),
                        perf_mode=DR)
            ot = ost.tile([P, M], fp32, tag="ot", name=f"ot_{ht}")
            for mb in range(2):
                ms = slice(mb * 512, (mb + 1) * 512)
                nc.vector.tensor_scalar(ot[:, ms], pss[mb],
                                        ab["al4"][:, ht:ht + 1], None, op0=op.mult)
                nc.sync.dma_start(out=out_d[:, ht, ms], in_=ot[:, ms])

    nc.compile()
    return nc


_NC_CACHE = {}


def kernel(x, w1, w2, w3, w4, bn_scale1, bn_bias1, bn_scale2, bn_bias2,
           bn_scale3, bn_bias3, act_scale1, act_scale2, act_scale3,
           _trace=False, _tmpdir=None):
    from concourse import mybir
    from concourse.bass_utils import run_bass_kernel_spmd

    np16 = mybir.dt.np(mybir.dt.float16)
    np8 = mybir.dt.np(mybir.dt.float8e4)

    # ---- host-side prep (exact fp32 replication of the quantizers) ----
    q1, s1 = _quant_int(w1)
    q2, s2 = _quant_int(w2)
    q3, s3 = _quant_int(w3)
    q4, s4 = _quant_int(w4)

    as1 = f32(np.asarray(act_scale1).reshape(-1)[0])
    as2 = f32(np.asarray(act_scale2).reshape(-1)[0])
    as3 = f32(np.asarray(act_scale3).reshape(-1)[0])

    al1 = (s1 * np.asarray(bn_scale1, f32) / as1).astype(f32)
    be1 = (np.asarray(bn_bias1, f32) / as1).astype(f32)
    al2 = (s2 * as1 * np.asarray(bn_scale2, f32) / as2).astype(f32)
    be2 = (np.asarray(bn_bias2, f32) / as2).astype(f32)
    al3 = (s3 * as2 * np.asarray(bn_scale3, f32) / as3).astype(f32)
    be3 = (np.asarray(bn_bias3, f32) / as3).astype(f32)
    al4 = np.full((CPAD,), s4 * as3, f32)

    q4p = np.zeros((CPAD, H), f32)
    q4p[:C] = q4

    w1p = _w_prep(q1, D // P, H // P, np16)
    w2p = _w_prep(q2, H // P, H // P, np8)
    w3p = _w_prep(q3, H // P, H // P, np8)
    w4p = _w_prep(q4p, H // P, CPAD // P, np8)

    x = np.asarray(x, f32)
    x_hi = x.astype(np16)
    x_lo = (x - x_hi.astype(f32)).astype(np16)

    shared = {
        "w1q": w1p, "w2q": w2p, "w3q": w3p, "w4q": w4p,
        "al1": _per_part(al1, H // P), "be1": _per_part(be1, H // P),
        "al2": _per_part(al2, H // P), "be2": _per_part(be2, H // P),
        "al3": _per_part(al3, H // P), "be3": _per_part(be3, H // P),
        "al4": _per_part(al4, CPAD // P),
    }
    in_maps = []
    for c in range(NCORES):
        rows = slice(c * M, (c + 1) * M)
        in_maps.append({
            **shared,
            "xh": _feat_major(x_hi[rows].T.astype(f32), D // P).astype(np16),
            "xl": _feat_major(x_lo[rows].T.astype(f32), D // P).astype(np16),
        })

    if "nc" not in _NC_CACHE:
        _NC_CACHE["nc"] = _build_bass()
    nc = _NC_CACHE["nc"]

    res = run_bass_kernel_spmd(nc, in_maps, core_ids=list(range(NCORES)),
                               trace=_trace, tmpdir=_tmpdir)
    outs = []
    for c in range(NCORES):
        o = np.asarray(res.results[c]["out"])          # [P, HT4, M]
        z = o.transpose(1, 0, 2).reshape(CPAD, M)[:C]  # [1000, M]
        outs.append(z.T)                               # [M, 1000]
    full = np.concatenate(outs, axis=0).astype(f32)
    if _trace:
        return full, res
    return full

